# revision 53
# baseline (speedup 1.0000x reference)
"""GIN message-passing encoder (3 layers) on 8 Trainium2 NeuronCores.

Problem: x_{l+1} = relu(BN(relu((x + agg(x)) @ W1 + b1) @ W2 + b2)),
agg[b, d] = sum over edges (s -> d) of x[b, s]; output = stack of the 3
layer outputs, shape [3, 16, 1024, 256].

Strategy
--------
- Data parallel over batch: B=16 split as 2 batch elements per core.
- The scatter-add is a dense matmul against a host-built (N x N) matrix
  Bm[s, d] = I[s, d] + multiplicity(edge s -> d); Bm entries are small
  ints {0,1,2}, exactly representable in fp8e4m3.
- Step 1 (the dominant 1024x1024 contraction) runs in fp8 with
  perf_mode=DoubleRow: stationary [128, 2, 128] + moving [128, 2, 512]
  contract 256 source nodes per matmul, halving PE cycles. x is
  quantized to fp8 at scale 1/4 (host for layer 0, GPSIMD on device for
  layers 1-2); the 4x is folded into W1 on the host. The two
  column-halves of the output accumulate in two interleaved PSUM banks
  sharing each stationary tile, so LDWEIGHTS hides under the previous
  matmul's stream.
- Steps 2/3 (the MLP) run bf16 x bf16 -> f32 PSUM (same PE rate as
  f32r, but FWL makes the per-matmul LDWEIGHTS hideable).
- Eval-mode BatchNorm is folded into W2/b2 on the host.
- x is quantized to fp8 on-device by Vector (layers 1-2; layer 0 on
  the host); post-ops are spread over Scalar (m0t copies except ft1-h1,
  step2 bias+relu, step3 relu for odd tiles) and Vector (m0t copy
  ft1-h1, step3 bias add, step3 relu for even tiles, fp8 requantize);
  the last block folds the b2 add into per-group bias-row matmuls so
  its tail is just relu + DMA.
- DRAM layouts are partition-major on both ends (host pre/post
  permutes) so every DMA moves multi-KB contiguous runs; initial loads
  and the final output tiles are split across the Sync and Scalar
  HWDGE queues.
"""

import os

import numpy as np

BN_EPS = 1e-5

B, N, F = 16, 1024, 256
L = 3
NCORES = 8
BPC = B // NCORES  # batch elements per core
P = 128
NT = N // P  # 8 node tiles
FT = F // P  # 2 feature tiles
HALF = 512   # psum bank free size (f32)
NH = N // HALF  # 2 halves of the node dim
XSCALE = 0.25  # fp8 quantization scale for x (folded into W1 on host)

_cache: dict = {}


def _build_nc():
    import concourse.bacc as bacc
    import concourse.mybir as mybir
    import concourse.tile as tile

    F32 = mybir.dt.float32
    F32R = mybir.dt.float32r
    BF16 = mybir.dt.bfloat16
    FP8 = mybir.dt.float8e4
    Relu = mybir.ActivationFunctionType.Relu
    Copy = mybir.ActivationFunctionType.Copy
    Alu = mybir.AluOpType
    DR = mybir.MatmulPerfMode.DoubleRow

    nc = bacc.Bacc()

    # All inputs are host-preswizzled to partition-major layouts so every
    # DMA reads multi-KB contiguous DRAM runs per partition.
    x0_d = nc.dram_tensor("x0", [BPC, P, NT, F], FP8, kind="ExternalInput")
    bm_d = nc.dram_tensor("bm", [NH, P, NT, HALF], FP8, kind="ExternalInput")
    w1_d = nc.dram_tensor("w1", [L, P, FT, F], BF16, kind="ExternalInput")
    w2_d = nc.dram_tensor("w2", [L, P, FT, F], BF16, kind="ExternalInput")
    b1_d = nc.dram_tensor("b1", [P, L * FT], F32, kind="ExternalInput")
    b2_d = nc.dram_tensor("b2", [P, L, HALF], F32R, kind="ExternalInput")
    ones_d = nc.dram_tensor("ones", [1, P], F32R, kind="ExternalInput")
    # Partition-major output layout: each partition owns contiguous
    # [NT, F] runs per (l, b), so output DMAs move 4 KB contiguous DRAM
    # chunks per partition (the host un-permutes afterwards).
    out_d = nc.dram_tensor("out", [L, BPC, P, NT, F], F32R,
                           kind="ExternalOutput")

    with tile.TileContext(nc) as tc:
        with (
            tc.tile_pool(name="const", bufs=1) as cpool,
            tc.tile_pool(name="xq", bufs=2) as xqpool,
            tc.tile_pool(name="work", bufs=3) as wpool,
            tc.tile_pool(name="yt", bufs=6) as ypool,
            tc.tile_pool(name="xf", bufs=4) as xfpool,
            tc.tile_pool(name="pm0", bufs=3, space="PSUM") as pm0,
            tc.tile_pool(name="ph1", bufs=2, space="PSUM") as ph1,
            tc.tile_pool(name="py", bufs=3, space="PSUM") as py,
        ):
            bq_sb = [
                cpool.tile([P, NT, HALF], FP8, name=f"bq_h{h}")
                for h in range(NH)
            ]
            w1_sb = cpool.tile([P, L, FT, F], BF16)
            w2_sb = cpool.tile([P, L, FT, F], BF16)
            b1_sb = cpool.tile([P, L * FT], F32)
            b2_sb = cpool.tile([P, L, HALF], F32R)
            ones_sb = cpool.tile([1, P], F32R)

            xq_cur = [
                xqpool.tile([P, NT, F], FP8, tag=f"xq{b}", name=f"xq_in{b}")
                for b in range(BPC)
            ]

            # Initial loads, split across the two HWDGE queues (Sync and
            # Scalar) so issue overhead parallelizes. Criticality order:
            # the first step-1 group needs Bm's half-0 columns + batch-0 x.
            # Sync's queue starts ~2 us before Scalar's, so it carries the
            # critical path: batch-0 x, then Bm half 0 in k-chunks so the
            # first accumulation group can start as soon as chunk 0 lands.
            # The critical first matmul needs xq0 + bm half 0: give them
            # the DMA fabric exclusively (only tiny b1/b2 pulls compete),
            # then stream the rest in need order.
            nc.sync.dma_start(xq_cur[0][:], x0_d[0])
            nc.sync.dma_start(bq_sb[0][:], bm_d[0])
            nc.scalar.dma_start(b1_sb[:], b1_d[:])
            nc.scalar.dma_start(b2_sb[:], b2_d[:])
            nc.scalar.dma_start(ones_sb[:], ones_d[:])
            nc.scalar.dma_start(xq_cur[1][:], x0_d[1])
            nc.scalar.dma_start(w1_sb[:, 0], w1_d[0])
            nc.scalar.dma_start(w2_sb[:, 0], w2_d[0])
            nc.sync.dma_start(bq_sb[1][:], bm_d[1])
            for l in range(1, L):
                nc.sync.dma_start(w1_sb[:, l], w1_d[l])
                nc.sync.dma_start(w2_sb[:, l], w2_d[l])


            def step1(l, b):
                # ---- step 1: m0T = (A + I) @ x / 4, fp8 DoubleRow ----
                m0t = wpool.tile([P, FT, N], BF16, tag=f"m0t{b}",
                                 name=f"m0t{l}{b}")
                if l == 0 and b == 0:
                    # Startup: only Bm's half-0 columns have arrived; run
                    # the halves sequentially (LDWEIGHTS exposed, but this
                    # window is DMA-bound and cold-clocked anyway).
                    for half in range(NH):
                        for ft in range(FT):
                            ps = pm0.tile([P, HALF], F32, tag="pm0",
                                          name=f"psa{half}{ft}")
                            for k2 in range(0, NT, 2):
                                nc.tensor.matmul(
                                    ps[:],
                                    xq_cur[b][:, k2:k2 + 2,
                                              ft * P:(ft + 1) * P],
                                    bq_sb[half][:, k2:k2 + 2, :],
                                    start=(k2 == 0),
                                    stop=(k2 == NT - 2),
                                    perf_mode=DR,
                                )
                            nc.scalar.activation(
                                m0t[:, ft, half * HALF:(half + 1) * HALF],
                                ps[:], Copy,
                            )
                    return m0t
                # Steady state: the two column-halves accumulate in two
                # interleaved PSUM banks sharing each stationary tile, so
                # every LDWEIGHTS hides under the previous matmul's stream.
                for ft in range(FT):
                    psh = [
                        pm0.tile([P, HALF], F32, tag="pm0",
                                 name=f"ps1_{l}{b}{ft}{h}")
                        for h in range(NH)
                    ]
                    for k2 in range(0, NT, 2):
                        for half in range(NH):
                            nc.tensor.matmul(
                                psh[half][:],
                                xq_cur[b][:, k2:k2 + 2, ft * P:(ft + 1) * P],
                                bq_sb[half][:, k2:k2 + 2, :],
                                start=(k2 == 0),
                                stop=(k2 == NT - 2),
                                perf_mode=DR,
                            )
                    for half in range(NH):
                        # The (ft1, h0) copy gates step2's third matmul;
                        # Scalar's queue is free here while Vector may
                        # still be draining the previous batch's step-3
                        # work, so only (ft1, h1) goes to Vector.
                        if ft == 0 or half == 0:
                            nc.scalar.activation(
                                m0t[:, ft, half * HALF:(half + 1) * HALF],
                                psh[half][:], Copy,
                            )
                        else:
                            nc.vector.tensor_copy(
                                m0t[:, ft, half * HALF:(half + 1) * HALF],
                                psh[half][:],
                            )
                return m0t

            def step2(l, b, m0t):
                # ---- step 2: h1T = relu(W1^T-contract @ m0T + b1) ----
                # W1 stationary tiles are shared by the two halves via
                # interleaved PSUM banks.
                h1t = wpool.tile([P, FT, N], BF16, tag=f"h1t{b}",
                                 name=f"h1t{l}{b}")
                for gt in range(FT):
                    psh = [
                        ph1.tile([P, HALF], F32, tag="ph1",
                                 name=f"ps2_{l}{b}{gt}{h}")
                        for h in range(NH)
                    ]
                    for fk in range(FT):
                        for half in range(NH):
                            nc.tensor.matmul(
                                psh[half][:],
                                w1_sb[:, l, fk, gt * P:(gt + 1) * P],
                                m0t[:, fk, half * HALF:(half + 1) * HALF],
                                start=(fk == 0),
                                stop=(fk == FT - 1),
                            )
                    for half in range(NH):
                        nc.scalar.activation(
                            h1t[:, gt, half * HALF:(half + 1) * HALF],
                            psh[half][:],
                            Relu,
                            bias=b1_sb[:, l * FT + gt:l * FT + gt + 1],
                        )
                return h1t

            def step3(l, b, h1t, xq_next):
                # ---- step 3: y = h1 @ W2' + b2', relu -> out + fp8 x ----
                last = (l == L - 1 and b == BPC - 1)
                xf = None
                for tp in range(NT // 2):
                    if tp % 2 == 0:
                        xf = xfpool.tile([P, 4, F], F32R, tag="xf",
                                         name=f"xf{l}{b}{tp}")
                    ps = py.tile([P, 2, F], F32, tag="py")
                    for j in range(2):
                        nt = 2 * tp + j
                        for gk in range(FT):
                            nc.tensor.matmul(
                                ps[:, j, :],
                                h1t[:, gk, nt * P:(nt + 1) * P],
                                w2_sb[:, l, gk, :],
                                start=(gk == 0),
                                stop=(gk == FT - 1) and not last,
                            )
                        if last:
                            # Tensor is idle at the end: fold the b2' add
                            # into each j's accumulation group via a 1-row
                            # matmul so the tail is just relu + DMA.
                            nc.tensor.matmul(
                                ps[:, j, :],
                                ones_sb[:],
                                b2_sb[0:1, l, j * F:(j + 1) * F],
                                start=False,
                                stop=True,
                            )
                    xh = xf[:, (tp % 2) * 2:(tp % 2) * 2 + 2, :]
                    if last:
                        if tp % 2 == 0:
                            nc.vector.tensor_scalar(
                                xh, ps[:], 0.0, None, op0=Alu.max,
                            )
                        else:
                            nc.scalar.activation(xh, ps[:], Relu)
                    else:
                        ytmp = ypool.tile([P, 2, F], F32, tag="ytmp")
                        nc.vector.scalar_tensor_tensor(
                            ytmp[:],
                            ps[:],
                            1.0,
                            b2_sb[:, l, :].rearrange("p (a f) -> p a f", a=2),
                            op0=Alu.mult,
                            op1=Alu.add,
                        )
                        if tp % 2 == 0:
                            nc.vector.tensor_scalar(
                                xh, ytmp[:], 0.0, None, op0=Alu.max,
                            )
                        else:
                            nc.scalar.activation(xh, ytmp[:], Relu)
                        if xq_next is not None:
                            nc.vector.tensor_scalar(
                                xq_next[b][:, 2 * tp:2 * tp + 2, :],
                                ytmp[:],
                                0.0, XSCALE, op0=Alu.max, op1=Alu.mult,
                            )
                    if last:
                        # Tail: per-tile DMAs fanned over both HWDGE
                        # queues so the final transfer is small and
                        # starts as early as possible.
                        eng = nc.scalar if tp % 2 == 1 else nc.sync
                        eng.dma_start(
                            out_d[l, b, :, 2 * tp:2 * tp + 2, :],
                            xf[:, (tp % 2) * 2:(tp % 2) * 2 + 2, :],
                        )
                    elif tp % 2 == 1:
                        nc.sync.dma_start(
                            out_d[l, b, :, (tp - 1) * 2:(tp + 1) * 2, :],
                            xf[:],
                        )

            for l in range(L):
                if l < L - 1:
                    xq_next = [
                        xqpool.tile([P, NT, F], FP8, tag=f"xq{b}",
                                    name=f"xq_n{l}{b}")
                        for b in range(BPC)
                    ]
                else:
                    xq_next = None
                # Per-batch stage order: consecutive stages use different
                # PSUM pools, so each pool gets a full stage to drain while
                # the other batch's matmuls keep the tensor engine fed.
                for b in range(BPC):
                    m0t = step1(l, b)
                    h1t = step2(l, b, m0t)
                    step3(l, b, h1t, xq_next)
                if xq_next is not None:
                    xq_cur = xq_next

    nc.finalize()
    return nc


def kernel(h, edge_index, W1, b1, W2, b2, gamma, beta, run_mean, run_var):
    import ml_dtypes
    from concourse.bass_utils import run_bass_kernel_spmd

    h = np.asarray(h, dtype=np.float32)
    edge_index = np.asarray(edge_index)
    W1 = np.asarray(W1, dtype=np.float32)
    b1 = np.asarray(b1, dtype=np.float32)
    W2 = np.asarray(W2, dtype=np.float32)
    b2 = np.asarray(b2, dtype=np.float32)
    gamma = np.asarray(gamma, dtype=np.float32)
    beta = np.asarray(beta, dtype=np.float32)
    run_mean = np.asarray(run_mean, dtype=np.float32)
    run_var = np.asarray(run_var, dtype=np.float32)

    # host-side preprocessing
    src = edge_index[0].astype(np.int64)
    dst = edge_index[1].astype(np.int64)
    bm = np.zeros((N, N), dtype=np.float32)
    np.add.at(bm, (src, dst), 1.0)
    bm[np.arange(N), np.arange(N)] += 1.0
    bmq = bm.astype(ml_dtypes.float8_e4m3)  # small ints: exact
    # partition-major: bmr[h, p, c, dh] = bm[c*P + p, h*HALF + dh]
    bmr = np.ascontiguousarray(
        bmq.reshape(NT, P, NH, HALF).transpose(2, 1, 0, 3)
    )

    inv = (gamma / np.sqrt(run_var + BN_EPS)).astype(np.float32)      # [L, F]
    w1s = (W1 / XSCALE).astype(ml_dtypes.bfloat16)                    # [L, F, F]
    w2f = (W2 * inv[:, None, :]).astype(ml_dtypes.bfloat16)           # [L, F, F]
    # partition-major: w[l, p, c, g] = w[l, c*P + p, g]
    w1r = np.ascontiguousarray(
        w1s.reshape(L, FT, P, F).transpose(0, 2, 1, 3)
    )
    w2r = np.ascontiguousarray(
        w2f.reshape(L, FT, P, F).transpose(0, 2, 1, 3)
    )
    b2f = (b2 * inv + beta - run_mean * inv).astype(np.float32)       # [L, F]

    # b1 as per-partition scalars: [P, L*FT], column l*FT+gt = b1[l, gt*128:...]
    b1r = np.ascontiguousarray(
        b1.reshape(L, FT, P).transpose(2, 0, 1).reshape(P, L * FT)
    )
    # b2' broadcast across partitions, twice along free (for [P, 2, F] pairs)
    b2r = np.ascontiguousarray(
        np.broadcast_to(
            np.concatenate([b2f, b2f], axis=1)[None], (P, L, HALF)
        )
    )

    if "nc" not in _cache:
        _cache["nc"] = _build_nc()
    nc = _cache["nc"]

    in_maps = []
    for c in range(NCORES):
        x0q = (h[c * BPC:(c + 1) * BPC] * XSCALE).astype(
            ml_dtypes.float8_e4m3
        )
        # partition-major: x0r[b, p, c, f] = x0q[b, c*P + p, f]
        x0r = np.ascontiguousarray(
            x0q.reshape(BPC, NT, P, F).transpose(0, 2, 1, 3)
        )
        in_maps.append({
            "x0": x0r,
            "bm": bmr,
            "w1": w1r,
            "w2": w2r,
            "b1": b1r,
            "b2": b2r,
            "ones": np.ones((1, P), dtype=np.float32),
        })

    trace = os.environ.get("KERNEL_TRACE") == "1"
    res = run_bass_kernel_spmd(
        nc, in_maps, core_ids=list(range(NCORES)), trace=trace
    )
    _cache["last_results"] = res
    # out comes back as [L, BPC, P, NT, F]; un-permute the node dim
    # (n = t * P + p) back to [L, BPC, N, F].
    outs = [
        np.ascontiguousarray(r["out"].transpose(0, 1, 3, 2, 4))
        .reshape(L, BPC, N, F)
        for r in res.results
    ]
    return np.concatenate(outs, axis=1)


# revision 54
# speedup vs baseline: 1.0434x; 1.0434x over previous
"""GIN message-passing encoder (3 layers) on 8 Trainium2 NeuronCores.

Problem: x_{l+1} = relu(BN(relu((x + agg(x)) @ W1 + b1) @ W2 + b2)),
agg[b, d] = sum over edges (s -> d) of x[b, s]; output = stack of the 3
layer outputs, shape [3, 16, 1024, 256].

Strategy
--------
- Data parallel over batch: B=16 split as 2 batch elements per core.
- The scatter-add is a dense matmul against a host-built (N x N) matrix
  Bm[s, d] = I[s, d] + multiplicity(edge s -> d); Bm entries are small
  ints {0,1,2}, exactly representable in fp8e4m3.
- Step 1 (the dominant 1024x1024 contraction) runs in fp8 with
  perf_mode=DoubleRow: stationary [128, 2, 128] + moving [128, 2, 512]
  contract 256 source nodes per matmul, halving PE cycles. x is
  quantized to fp8 at scale 1/4 (host for layer 0, GPSIMD on device for
  layers 1-2); the 4x is folded into W1 on the host. The two
  column-halves of the output accumulate in two interleaved PSUM banks
  sharing each stationary tile, so LDWEIGHTS hides under the previous
  matmul's stream.
- Steps 2/3 (the MLP) run bf16 x bf16 -> f32 PSUM (same PE rate as
  f32r, but FWL makes the per-matmul LDWEIGHTS hideable).
- Eval-mode BatchNorm is folded into W2/b2 on the host.
- x is quantized to fp8 on-device by Vector (layers 1-2; layer 0 on
  the host); post-ops are spread over Scalar (m0t copies except ft1-h1,
  step2 bias+relu, step3 relu for odd tiles) and Vector (m0t copy
  ft1-h1, step3 bias add, step3 relu for even tiles, fp8 requantize);
  the last block folds the b2 add into per-group bias-row matmuls so
  its tail is just relu + DMA.
- DRAM layouts are partition-major on both ends (host pre/post
  permutes) so every DMA moves multi-KB contiguous runs; initial loads
  and the final output tiles are split across the Sync and Scalar
  HWDGE queues.
"""

import os

import numpy as np

BN_EPS = 1e-5

B, N, F = 16, 1024, 256
L = 3
NCORES = 8
BPC = B // NCORES  # batch elements per core
P = 128
NT = N // P  # 8 node tiles
FT = F // P  # 2 feature tiles
HALF = 512   # psum bank free size (f32)
NH = N // HALF  # 2 halves of the node dim
XSCALE = 0.25  # fp8 quantization scale for x (folded into W1 on host)

_cache: dict = {}


def _build_nc():
    import concourse.bacc as bacc
    import concourse.mybir as mybir
    import concourse.tile as tile

    F32 = mybir.dt.float32
    F32R = mybir.dt.float32r
    BF16 = mybir.dt.bfloat16
    FP8 = mybir.dt.float8e4
    Relu = mybir.ActivationFunctionType.Relu
    Copy = mybir.ActivationFunctionType.Copy
    Alu = mybir.AluOpType
    DR = mybir.MatmulPerfMode.DoubleRow

    nc = bacc.Bacc()

    # All inputs are host-preswizzled to partition-major layouts so every
    # DMA reads multi-KB contiguous DRAM runs per partition.
    x0_d = nc.dram_tensor("x0", [BPC, P, NT, F], FP8, kind="ExternalInput")
    bm_d = nc.dram_tensor("bm", [NH, P, NT, HALF], FP8, kind="ExternalInput")
    w1_d = nc.dram_tensor("w1", [L, P, FT, F], BF16, kind="ExternalInput")
    w2_d = nc.dram_tensor("w2", [L, P, FT, F], BF16, kind="ExternalInput")
    b1_d = nc.dram_tensor("b1", [P, L * FT], F32, kind="ExternalInput")
    b2_d = nc.dram_tensor("b2", [P, L, HALF], F32R, kind="ExternalInput")
    ones_d = nc.dram_tensor("ones", [1, P], F32R, kind="ExternalInput")
    # Partition-major output layout: each partition owns contiguous
    # [NT, F] runs per (l, b), so output DMAs move 4 KB contiguous DRAM
    # chunks per partition (the host un-permutes afterwards).
    out_d = nc.dram_tensor("out", [L, BPC, P, NT, F], F32R,
                           kind="ExternalOutput")

    with tile.TileContext(nc) as tc:
        with (
            tc.tile_pool(name="const", bufs=1) as cpool,
            tc.tile_pool(name="xq", bufs=2) as xqpool,
            tc.tile_pool(name="work", bufs=3) as wpool,
            tc.tile_pool(name="yt", bufs=6) as ypool,
            tc.tile_pool(name="xf", bufs=4) as xfpool,
            tc.tile_pool(name="pm0", bufs=3, space="PSUM") as pm0,
            tc.tile_pool(name="ph1", bufs=2, space="PSUM") as ph1,
            tc.tile_pool(name="py", bufs=3, space="PSUM") as py,
        ):
            bq_sb = [
                cpool.tile([P, NT, HALF], FP8, name=f"bq_h{h}")
                for h in range(NH)
            ]
            w1_sb = cpool.tile([P, L, FT, F], BF16)
            w2_sb = cpool.tile([P, L, FT, F], BF16)
            b1_sb = cpool.tile([P, L * FT], F32)
            b2_sb = cpool.tile([P, L, HALF], F32R)
            ones_sb = cpool.tile([1, P], F32R)

            xq_cur = [
                xqpool.tile([P, NT, F], FP8, tag=f"xq{b}", name=f"xq_in{b}")
                for b in range(BPC)
            ]

            # Initial loads, split across the two HWDGE queues (Sync and
            # Scalar) so issue overhead parallelizes. Criticality order:
            # the first step-1 group needs Bm's half-0 columns + batch-0 x.
            # Sync's queue starts ~2 us before Scalar's, so it carries the
            # critical path: batch-0 x, then Bm half 0 in k-chunks so the
            # first accumulation group can start as soon as chunk 0 lands.
            # The critical first matmul needs xq0 + bm half 0: give them
            # the DMA fabric exclusively (only tiny b1/b2 pulls compete),
            # then stream the rest in need order.
            nc.sync.dma_start(xq_cur[0][:], x0_d[0])
            nc.sync.dma_start(bq_sb[0][:], bm_d[0])
            nc.scalar.dma_start(b1_sb[:], b1_d[:])
            nc.scalar.dma_start(b2_sb[:], b2_d[:])
            nc.scalar.dma_start(ones_sb[:], ones_d[:])
            nc.scalar.dma_start(xq_cur[1][:], x0_d[1])
            nc.scalar.dma_start(w1_sb[:, 0], w1_d[0])
            nc.scalar.dma_start(w2_sb[:, 0], w2_d[0])
            nc.sync.dma_start(bq_sb[1][:], bm_d[1])
            for l in range(1, L):
                nc.sync.dma_start(w1_sb[:, l], w1_d[l])
                nc.sync.dma_start(w2_sb[:, l], w2_d[l])


            def step1(l, b):
                # ---- step 1: m0T = (A + I) @ x / 4, fp8 DoubleRow ----
                m0t = wpool.tile([P, FT, N], BF16, tag=f"m0t{b}",
                                 name=f"m0t{l}{b}")
                if l == 0 and b == 0:
                    # Startup: only Bm's half-0 columns have arrived; run
                    # the halves sequentially (LDWEIGHTS exposed, but this
                    # window is DMA-bound and cold-clocked anyway).
                    for half in range(NH):
                        for ft in range(FT):
                            ps = pm0.tile([P, HALF], F32, tag="pm0",
                                          name=f"psa{half}{ft}")
                            for k2 in range(0, NT, 2):
                                nc.tensor.matmul(
                                    ps[:],
                                    xq_cur[b][:, k2:k2 + 2,
                                              ft * P:(ft + 1) * P],
                                    bq_sb[half][:, k2:k2 + 2, :],
                                    start=(k2 == 0),
                                    stop=(k2 == NT - 2),
                                    perf_mode=DR,
                                )
                            nc.scalar.activation(
                                m0t[:, ft, half * HALF:(half + 1) * HALF],
                                ps[:], Copy,
                            )
                    return m0t
                # Steady state: the two column-halves accumulate in two
                # interleaved PSUM banks sharing each stationary tile, so
                # every LDWEIGHTS hides under the previous matmul's stream.
                for ft in range(FT):
                    psh = [
                        pm0.tile([P, HALF], F32, tag="pm0",
                                 name=f"ps1_{l}{b}{ft}{h}")
                        for h in range(NH)
                    ]
                    for k2 in range(0, NT, 2):
                        for half in range(NH):
                            nc.tensor.matmul(
                                psh[half][:],
                                xq_cur[b][:, k2:k2 + 2, ft * P:(ft + 1) * P],
                                bq_sb[half][:, k2:k2 + 2, :],
                                start=(k2 == 0),
                                stop=(k2 == NT - 2),
                                perf_mode=DR,
                            )
                    for half in range(NH):
                        # The (ft1, h0) copy gates step2's third matmul;
                        # Scalar's queue is free here while Vector may
                        # still be draining the previous batch's step-3
                        # work, so only (ft1, h1) goes to Vector.
                        if ft == 0 or half == 0:
                            nc.scalar.activation(
                                m0t[:, ft, half * HALF:(half + 1) * HALF],
                                psh[half][:], Copy,
                            )
                        else:
                            nc.vector.tensor_copy(
                                m0t[:, ft, half * HALF:(half + 1) * HALF],
                                psh[half][:],
                            )
                return m0t

            def step2(l, b, m0t):
                # ---- step 2: h1T = relu(W1^T-contract @ m0T + b1) ----
                # W1 stationary tiles are shared by the two halves via
                # interleaved PSUM banks.
                h1t = wpool.tile([P, FT, N], BF16, tag=f"h1t{b}",
                                 name=f"h1t{l}{b}")
                for gt in range(FT):
                    psh = [
                        ph1.tile([P, HALF], F32, tag="ph1",
                                 name=f"ps2_{l}{b}{gt}{h}")
                        for h in range(NH)
                    ]
                    for fk in range(FT):
                        for half in range(NH):
                            nc.tensor.matmul(
                                psh[half][:],
                                w1_sb[:, l, fk, gt * P:(gt + 1) * P],
                                m0t[:, fk, half * HALF:(half + 1) * HALF],
                                start=(fk == 0),
                                stop=(fk == FT - 1),
                            )
                    for half in range(NH):
                        nc.scalar.activation(
                            h1t[:, gt, half * HALF:(half + 1) * HALF],
                            psh[half][:],
                            Relu,
                            bias=b1_sb[:, l * FT + gt:l * FT + gt + 1],
                        )
                return h1t

            def step3(l, b, h1t, xq_next):
                # ---- step 3: y = h1 @ W2' + b2', relu -> out + fp8 x ----
                last = (l == L - 1 and b == BPC - 1)
                xf = None
                for tp in range(NT // 2):
                    if tp % 2 == 0:
                        xf = xfpool.tile([P, 4, F], F32R, tag="xf",
                                         name=f"xf{l}{b}{tp}")
                    ps = py.tile([P, 2, F], F32, tag="py")
                    for j in range(2):
                        nt = 2 * tp + j
                        for gk in range(FT):
                            nc.tensor.matmul(
                                ps[:, j, :],
                                h1t[:, gk, nt * P:(nt + 1) * P],
                                w2_sb[:, l, gk, :],
                                start=(gk == 0),
                                stop=(gk == FT - 1) and not last,
                            )
                        if last:
                            # Tensor is idle at the end: fold the b2' add
                            # into each j's accumulation group via a 1-row
                            # matmul so the tail is just relu + DMA.
                            nc.tensor.matmul(
                                ps[:, j, :],
                                ones_sb[:],
                                b2_sb[0:1, l, j * F:(j + 1) * F],
                                start=False,
                                stop=True,
                            )
                    xh = xf[:, (tp % 2) * 2:(tp % 2) * 2 + 2, :]
                    if last:
                        if tp % 2 == 0:
                            nc.vector.tensor_scalar(
                                xh, ps[:], 0.0, None, op0=Alu.max,
                            )
                        else:
                            nc.scalar.activation(xh, ps[:], Relu)
                    else:
                        ytmp = ypool.tile([P, 2, F], F32, tag="ytmp")
                        nc.vector.scalar_tensor_tensor(
                            ytmp[:],
                            ps[:],
                            1.0,
                            b2_sb[:, l, :].rearrange("p (a f) -> p a f", a=2),
                            op0=Alu.mult,
                            op1=Alu.add,
                        )
                        if tp % 2 == 0:
                            nc.vector.tensor_scalar(
                                xh, ytmp[:], 0.0, None, op0=Alu.max,
                            )
                        else:
                            nc.scalar.activation(xh, ytmp[:], Relu)
                        if xq_next is not None:
                            nc.vector.tensor_scalar(
                                xq_next[b][:, 2 * tp:2 * tp + 2, :],
                                ytmp[:],
                                0.0, XSCALE, op0=Alu.max, op1=Alu.mult,
                            )
                    if last:
                        # Tail: small per-tile DMAs, all on Sync (idle by
                        # now) so Scalar's relu consumers keep the py
                        # ring draining at full rate.
                        nc.sync.dma_start(
                            out_d[l, b, :, 2 * tp:2 * tp + 2, :],
                            xf[:, (tp % 2) * 2:(tp % 2) * 2 + 2, :],
                        )
                    elif tp % 2 == 1:
                        nc.sync.dma_start(
                            out_d[l, b, :, (tp - 1) * 2:(tp + 1) * 2, :],
                            xf[:],
                        )

            for l in range(L):
                if l < L - 1:
                    xq_next = [
                        xqpool.tile([P, NT, F], FP8, tag=f"xq{b}",
                                    name=f"xq_n{l}{b}")
                        for b in range(BPC)
                    ]
                else:
                    xq_next = None
                # Per-batch stage order: consecutive stages use different
                # PSUM pools, so each pool gets a full stage to drain while
                # the other batch's matmuls keep the tensor engine fed.
                for b in range(BPC):
                    m0t = step1(l, b)
                    h1t = step2(l, b, m0t)
                    step3(l, b, h1t, xq_next)
                if xq_next is not None:
                    xq_cur = xq_next

    nc.finalize()
    return nc


def kernel(h, edge_index, W1, b1, W2, b2, gamma, beta, run_mean, run_var):
    import ml_dtypes
    from concourse.bass_utils import run_bass_kernel_spmd

    h = np.asarray(h, dtype=np.float32)
    edge_index = np.asarray(edge_index)
    W1 = np.asarray(W1, dtype=np.float32)
    b1 = np.asarray(b1, dtype=np.float32)
    W2 = np.asarray(W2, dtype=np.float32)
    b2 = np.asarray(b2, dtype=np.float32)
    gamma = np.asarray(gamma, dtype=np.float32)
    beta = np.asarray(beta, dtype=np.float32)
    run_mean = np.asarray(run_mean, dtype=np.float32)
    run_var = np.asarray(run_var, dtype=np.float32)

    # host-side preprocessing
    src = edge_index[0].astype(np.int64)
    dst = edge_index[1].astype(np.int64)
    bm = np.zeros((N, N), dtype=np.float32)
    np.add.at(bm, (src, dst), 1.0)
    bm[np.arange(N), np.arange(N)] += 1.0
    bmq = bm.astype(ml_dtypes.float8_e4m3)  # small ints: exact
    # partition-major: bmr[h, p, c, dh] = bm[c*P + p, h*HALF + dh]
    bmr = np.ascontiguousarray(
        bmq.reshape(NT, P, NH, HALF).transpose(2, 1, 0, 3)
    )

    inv = (gamma / np.sqrt(run_var + BN_EPS)).astype(np.float32)      # [L, F]
    w1s = (W1 / XSCALE).astype(ml_dtypes.bfloat16)                    # [L, F, F]
    w2f = (W2 * inv[:, None, :]).astype(ml_dtypes.bfloat16)           # [L, F, F]
    # partition-major: w[l, p, c, g] = w[l, c*P + p, g]
    w1r = np.ascontiguousarray(
        w1s.reshape(L, FT, P, F).transpose(0, 2, 1, 3)
    )
    w2r = np.ascontiguousarray(
        w2f.reshape(L, FT, P, F).transpose(0, 2, 1, 3)
    )
    b2f = (b2 * inv + beta - run_mean * inv).astype(np.float32)       # [L, F]

    # b1 as per-partition scalars: [P, L*FT], column l*FT+gt = b1[l, gt*128:...]
    b1r = np.ascontiguousarray(
        b1.reshape(L, FT, P).transpose(2, 0, 1).reshape(P, L * FT)
    )
    # b2' broadcast across partitions, twice along free (for [P, 2, F] pairs)
    b2r = np.ascontiguousarray(
        np.broadcast_to(
            np.concatenate([b2f, b2f], axis=1)[None], (P, L, HALF)
        )
    )

    if "nc" not in _cache:
        _cache["nc"] = _build_nc()
    nc = _cache["nc"]

    in_maps = []
    for c in range(NCORES):
        x0q = (h[c * BPC:(c + 1) * BPC] * XSCALE).astype(
            ml_dtypes.float8_e4m3
        )
        # partition-major: x0r[b, p, c, f] = x0q[b, c*P + p, f]
        x0r = np.ascontiguousarray(
            x0q.reshape(BPC, NT, P, F).transpose(0, 2, 1, 3)
        )
        in_maps.append({
            "x0": x0r,
            "bm": bmr,
            "w1": w1r,
            "w2": w2r,
            "b1": b1r,
            "b2": b2r,
            "ones": np.ones((1, P), dtype=np.float32),
        })

    trace = os.environ.get("KERNEL_TRACE") == "1"
    res = run_bass_kernel_spmd(
        nc, in_maps, core_ids=list(range(NCORES)), trace=trace
    )
    _cache["last_results"] = res
    # out comes back as [L, BPC, P, NT, F]; un-permute the node dim
    # (n = t * P + p) back to [L, BPC, N, F].
    outs = [
        np.ascontiguousarray(r["out"].transpose(0, 1, 3, 2, 4))
        .reshape(L, BPC, N, F)
        for r in res.results
    ]
    return np.concatenate(outs, axis=1)


# revision 55
# speedup vs baseline: 1.0537x; 1.0099x over previous
"""GIN message-passing encoder (3 layers) on 8 Trainium2 NeuronCores.

Problem: x_{l+1} = relu(BN(relu((x + agg(x)) @ W1 + b1) @ W2 + b2)),
agg[b, d] = sum over edges (s -> d) of x[b, s]; output = stack of the 3
layer outputs, shape [3, 16, 1024, 256].

Strategy
--------
- Data parallel over batch: B=16 split as 2 batch elements per core.
- The scatter-add is a dense matmul against a host-built (N x N) matrix
  Bm[s, d] = I[s, d] + multiplicity(edge s -> d); Bm entries are small
  ints {0,1,2}, exactly representable in fp8e4m3.
- Step 1 (the dominant 1024x1024 contraction) runs in fp8 with
  perf_mode=DoubleRow: stationary [128, 2, 128] + moving [128, 2, 512]
  contract 256 source nodes per matmul, halving PE cycles. x is
  quantized to fp8 at scale 1/4 (host for layer 0, GPSIMD on device for
  layers 1-2); the 4x is folded into W1 on the host. The two
  column-halves of the output accumulate in two interleaved PSUM banks
  sharing each stationary tile, so LDWEIGHTS hides under the previous
  matmul's stream.
- Steps 2/3 (the MLP) run bf16 x bf16 -> f32 PSUM (same PE rate as
  f32r, but FWL makes the per-matmul LDWEIGHTS hideable).
- Eval-mode BatchNorm is folded into W2/b2 on the host.
- x is quantized to fp8 on-device by Vector (layers 1-2; layer 0 on
  the host); post-ops are spread over Scalar (m0t copies except ft1-h1,
  step2 bias+relu, step3 relu for odd tiles) and Vector (m0t copy
  ft1-h1, step3 bias add, step3 relu for even tiles, fp8 requantize);
  the last block folds the b2 add into per-group bias-row matmuls so
  its tail is just relu + DMA.
- DRAM layouts are partition-major on both ends (host pre/post
  permutes) so every DMA moves multi-KB contiguous runs; initial loads
  and the final output tiles are split across the Sync and Scalar
  HWDGE queues.
"""

import os

import numpy as np

BN_EPS = 1e-5

B, N, F = 16, 1024, 256
L = 3
NCORES = 8
BPC = B // NCORES  # batch elements per core
P = 128
NT = N // P  # 8 node tiles
FT = F // P  # 2 feature tiles
HALF = 512   # psum bank free size (f32)
NH = N // HALF  # 2 halves of the node dim
XSCALE = 0.25  # fp8 quantization scale for x (folded into W1 on host)

_cache: dict = {}


def _build_nc():
    import concourse.bacc as bacc
    import concourse.mybir as mybir
    import concourse.tile as tile

    F32 = mybir.dt.float32
    F32R = mybir.dt.float32r
    BF16 = mybir.dt.bfloat16
    FP8 = mybir.dt.float8e4
    Relu = mybir.ActivationFunctionType.Relu
    Copy = mybir.ActivationFunctionType.Copy
    Alu = mybir.AluOpType
    DR = mybir.MatmulPerfMode.DoubleRow

    nc = bacc.Bacc()

    # All inputs are host-preswizzled to partition-major layouts so every
    # DMA reads multi-KB contiguous DRAM runs per partition.
    x0_d = nc.dram_tensor("x0", [BPC, P, NT, F], FP8, kind="ExternalInput")
    bm_d = nc.dram_tensor("bm", [NH, P, NT, HALF], FP8, kind="ExternalInput")
    w1_d = nc.dram_tensor("w1", [L, P, FT, F], BF16, kind="ExternalInput")
    w2_d = nc.dram_tensor("w2", [L, P, FT, F], BF16, kind="ExternalInput")
    b1_d = nc.dram_tensor("b1", [P, L * FT], F32, kind="ExternalInput")
    b2_d = nc.dram_tensor("b2", [P, L, HALF], F32R, kind="ExternalInput")
    ones_d = nc.dram_tensor("ones", [1, P], F32R, kind="ExternalInput")
    # Partition-major output layout: each partition owns contiguous
    # [NT, F] runs per (l, b), so output DMAs move 4 KB contiguous DRAM
    # chunks per partition (the host un-permutes afterwards).
    out_d = nc.dram_tensor("out", [L, BPC, P, NT, F], F32R,
                           kind="ExternalOutput")

    with tile.TileContext(nc) as tc:
        with (
            tc.tile_pool(name="const", bufs=1) as cpool,
            tc.tile_pool(name="xq", bufs=2) as xqpool,
            tc.tile_pool(name="work", bufs=3) as wpool,
            tc.tile_pool(name="yt", bufs=9) as ypool,
            tc.tile_pool(name="xf", bufs=6) as xfpool,
            tc.tile_pool(name="pm0", bufs=3, space="PSUM") as pm0,
            tc.tile_pool(name="ph1", bufs=2, space="PSUM") as ph1,
            tc.tile_pool(name="py", bufs=3, space="PSUM") as py,
        ):
            bq_sb = [
                cpool.tile([P, NT, HALF], FP8, name=f"bq_h{h}")
                for h in range(NH)
            ]
            w1_sb = cpool.tile([P, L, FT, F], BF16)
            w2_sb = cpool.tile([P, L, FT, F], BF16)
            b1_sb = cpool.tile([P, L * FT], F32)
            b2_sb = cpool.tile([P, L, HALF], F32R)
            ones_sb = cpool.tile([1, P], F32R)

            xq_cur = [
                xqpool.tile([P, NT, F], FP8, tag=f"xq{b}", name=f"xq_in{b}")
                for b in range(BPC)
            ]

            # Initial loads, split across the two HWDGE queues (Sync and
            # Scalar) so issue overhead parallelizes. Criticality order:
            # the first step-1 group needs Bm's half-0 columns + batch-0 x.
            # Sync's queue starts ~2 us before Scalar's, so it carries the
            # critical path: batch-0 x, then Bm half 0 in k-chunks so the
            # first accumulation group can start as soon as chunk 0 lands.
            # The critical first matmul needs xq0 + bm half 0: give them
            # the DMA fabric exclusively (only tiny b1/b2 pulls compete),
            # then stream the rest in need order.
            nc.sync.dma_start(xq_cur[0][:], x0_d[0])
            nc.sync.dma_start(bq_sb[0][:], bm_d[0])
            nc.scalar.dma_start(b1_sb[:], b1_d[:])
            nc.scalar.dma_start(b2_sb[:], b2_d[:])
            nc.scalar.dma_start(ones_sb[:], ones_d[:])
            nc.scalar.dma_start(xq_cur[1][:], x0_d[1])
            nc.scalar.dma_start(w1_sb[:, 0], w1_d[0])
            nc.scalar.dma_start(w2_sb[:, 0], w2_d[0])
            nc.sync.dma_start(bq_sb[1][:], bm_d[1])
            for l in range(1, L):
                nc.sync.dma_start(w1_sb[:, l], w1_d[l])
                nc.sync.dma_start(w2_sb[:, l], w2_d[l])


            def step1(l, b):
                # ---- step 1: m0T = (A + I) @ x / 4, fp8 DoubleRow ----
                m0t = wpool.tile([P, FT, N], BF16, tag=f"m0t{b}",
                                 name=f"m0t{l}{b}")
                if l == 0 and b == 0:
                    # Startup: only Bm's half-0 columns have arrived; run
                    # the halves sequentially (LDWEIGHTS exposed, but this
                    # window is DMA-bound and cold-clocked anyway).
                    for half in range(NH):
                        for ft in range(FT):
                            ps = pm0.tile([P, HALF], F32, tag="pm0",
                                          name=f"psa{half}{ft}")
                            for k2 in range(0, NT, 2):
                                nc.tensor.matmul(
                                    ps[:],
                                    xq_cur[b][:, k2:k2 + 2,
                                              ft * P:(ft + 1) * P],
                                    bq_sb[half][:, k2:k2 + 2, :],
                                    start=(k2 == 0),
                                    stop=(k2 == NT - 2),
                                    perf_mode=DR,
                                )
                            nc.scalar.activation(
                                m0t[:, ft, half * HALF:(half + 1) * HALF],
                                ps[:], Copy,
                            )
                    return m0t
                # Steady state: the two column-halves accumulate in two
                # interleaved PSUM banks sharing each stationary tile, so
                # every LDWEIGHTS hides under the previous matmul's stream.
                for ft in range(FT):
                    psh = [
                        pm0.tile([P, HALF], F32, tag="pm0",
                                 name=f"ps1_{l}{b}{ft}{h}")
                        for h in range(NH)
                    ]
                    for k2 in range(0, NT, 2):
                        for half in range(NH):
                            nc.tensor.matmul(
                                psh[half][:],
                                xq_cur[b][:, k2:k2 + 2, ft * P:(ft + 1) * P],
                                bq_sb[half][:, k2:k2 + 2, :],
                                start=(k2 == 0),
                                stop=(k2 == NT - 2),
                                perf_mode=DR,
                            )
                    for half in range(NH):
                        # The (ft1, h0) copy gates step2's third matmul;
                        # Scalar's queue is free here while Vector may
                        # still be draining the previous batch's step-3
                        # work, so only (ft1, h1) goes to Vector.
                        if ft == 0 or half == 0:
                            nc.scalar.activation(
                                m0t[:, ft, half * HALF:(half + 1) * HALF],
                                psh[half][:], Copy,
                            )
                        else:
                            nc.vector.tensor_copy(
                                m0t[:, ft, half * HALF:(half + 1) * HALF],
                                psh[half][:],
                            )
                return m0t

            def step2(l, b, m0t):
                # ---- step 2: h1T = relu(W1^T-contract @ m0T + b1) ----
                # W1 stationary tiles are shared by the two halves via
                # interleaved PSUM banks.
                h1t = wpool.tile([P, FT, N], BF16, tag=f"h1t{b}",
                                 name=f"h1t{l}{b}")
                for gt in range(FT):
                    psh = [
                        ph1.tile([P, HALF], F32, tag="ph1",
                                 name=f"ps2_{l}{b}{gt}{h}")
                        for h in range(NH)
                    ]
                    for fk in range(FT):
                        for half in range(NH):
                            nc.tensor.matmul(
                                psh[half][:],
                                w1_sb[:, l, fk, gt * P:(gt + 1) * P],
                                m0t[:, fk, half * HALF:(half + 1) * HALF],
                                start=(fk == 0),
                                stop=(fk == FT - 1),
                            )
                    for half in range(NH):
                        nc.scalar.activation(
                            h1t[:, gt, half * HALF:(half + 1) * HALF],
                            psh[half][:],
                            Relu,
                            bias=b1_sb[:, l * FT + gt:l * FT + gt + 1],
                        )
                return h1t

            def step3(l, b, h1t, xq_next):
                # ---- step 3: y = h1 @ W2' + b2', relu -> out + fp8 x ----
                last = (l == L - 1 and b == BPC - 1)
                xf = None
                for tp in range(NT // 2):
                    if tp % 2 == 0:
                        xf = xfpool.tile([P, 4, F], F32R, tag="xf",
                                         name=f"xf{l}{b}{tp}")
                    ps = py.tile([P, 2, F], F32, tag="py")
                    for j in range(2):
                        nt = 2 * tp + j
                        for gk in range(FT):
                            nc.tensor.matmul(
                                ps[:, j, :],
                                h1t[:, gk, nt * P:(nt + 1) * P],
                                w2_sb[:, l, gk, :],
                                start=(gk == 0),
                                stop=(gk == FT - 1) and not last,
                            )
                        if last:
                            # Tensor is idle at the end: fold the b2' add
                            # into each j's accumulation group via a 1-row
                            # matmul so the tail is just relu + DMA.
                            nc.tensor.matmul(
                                ps[:, j, :],
                                ones_sb[:],
                                b2_sb[0:1, l, j * F:(j + 1) * F],
                                start=False,
                                stop=True,
                            )
                    xh = xf[:, (tp % 2) * 2:(tp % 2) * 2 + 2, :]
                    if last:
                        if tp % 2 == 0:
                            nc.vector.tensor_scalar(
                                xh, ps[:], 0.0, None, op0=Alu.max,
                            )
                        else:
                            nc.scalar.activation(xh, ps[:], Relu)
                    else:
                        ytmp = ypool.tile([P, 2, F], F32, tag="ytmp")
                        nc.vector.scalar_tensor_tensor(
                            ytmp[:],
                            ps[:],
                            1.0,
                            b2_sb[:, l, :].rearrange("p (a f) -> p a f", a=2),
                            op0=Alu.mult,
                            op1=Alu.add,
                        )
                        if tp % 2 == 0:
                            nc.vector.tensor_scalar(
                                xh, ytmp[:], 0.0, None, op0=Alu.max,
                            )
                        else:
                            nc.scalar.activation(xh, ytmp[:], Relu)
                        if xq_next is not None:
                            nc.vector.tensor_scalar(
                                xq_next[b][:, 2 * tp:2 * tp + 2, :],
                                ytmp[:],
                                0.0, XSCALE, op0=Alu.max, op1=Alu.mult,
                            )
                    if last:
                        # Tail: small per-tile DMAs, all on Sync (idle by
                        # now) so Scalar's relu consumers keep the py
                        # ring draining at full rate.
                        nc.sync.dma_start(
                            out_d[l, b, :, 2 * tp:2 * tp + 2, :],
                            xf[:, (tp % 2) * 2:(tp % 2) * 2 + 2, :],
                        )
                    elif tp % 2 == 1:
                        nc.sync.dma_start(
                            out_d[l, b, :, (tp - 1) * 2:(tp + 1) * 2, :],
                            xf[:],
                        )

            for l in range(L):
                if l < L - 1:
                    xq_next = [
                        xqpool.tile([P, NT, F], FP8, tag=f"xq{b}",
                                    name=f"xq_n{l}{b}")
                        for b in range(BPC)
                    ]
                else:
                    xq_next = None
                # Per-batch stage order: consecutive stages use different
                # PSUM pools, so each pool gets a full stage to drain while
                # the other batch's matmuls keep the tensor engine fed.
                for b in range(BPC):
                    m0t = step1(l, b)
                    h1t = step2(l, b, m0t)
                    step3(l, b, h1t, xq_next)
                if xq_next is not None:
                    xq_cur = xq_next

    nc.finalize()
    return nc


def kernel(h, edge_index, W1, b1, W2, b2, gamma, beta, run_mean, run_var):
    import ml_dtypes
    from concourse.bass_utils import run_bass_kernel_spmd

    h = np.asarray(h, dtype=np.float32)
    edge_index = np.asarray(edge_index)
    W1 = np.asarray(W1, dtype=np.float32)
    b1 = np.asarray(b1, dtype=np.float32)
    W2 = np.asarray(W2, dtype=np.float32)
    b2 = np.asarray(b2, dtype=np.float32)
    gamma = np.asarray(gamma, dtype=np.float32)
    beta = np.asarray(beta, dtype=np.float32)
    run_mean = np.asarray(run_mean, dtype=np.float32)
    run_var = np.asarray(run_var, dtype=np.float32)

    # host-side preprocessing
    src = edge_index[0].astype(np.int64)
    dst = edge_index[1].astype(np.int64)
    bm = np.zeros((N, N), dtype=np.float32)
    np.add.at(bm, (src, dst), 1.0)
    bm[np.arange(N), np.arange(N)] += 1.0
    bmq = bm.astype(ml_dtypes.float8_e4m3)  # small ints: exact
    # partition-major: bmr[h, p, c, dh] = bm[c*P + p, h*HALF + dh]
    bmr = np.ascontiguousarray(
        bmq.reshape(NT, P, NH, HALF).transpose(2, 1, 0, 3)
    )

    inv = (gamma / np.sqrt(run_var + BN_EPS)).astype(np.float32)      # [L, F]
    w1s = (W1 / XSCALE).astype(ml_dtypes.bfloat16)                    # [L, F, F]
    w2f = (W2 * inv[:, None, :]).astype(ml_dtypes.bfloat16)           # [L, F, F]
    # partition-major: w[l, p, c, g] = w[l, c*P + p, g]
    w1r = np.ascontiguousarray(
        w1s.reshape(L, FT, P, F).transpose(0, 2, 1, 3)
    )
    w2r = np.ascontiguousarray(
        w2f.reshape(L, FT, P, F).transpose(0, 2, 1, 3)
    )
    b2f = (b2 * inv + beta - run_mean * inv).astype(np.float32)       # [L, F]

    # b1 as per-partition scalars: [P, L*FT], column l*FT+gt = b1[l, gt*128:...]
    b1r = np.ascontiguousarray(
        b1.reshape(L, FT, P).transpose(2, 0, 1).reshape(P, L * FT)
    )
    # b2' broadcast across partitions, twice along free (for [P, 2, F] pairs)
    b2r = np.ascontiguousarray(
        np.broadcast_to(
            np.concatenate([b2f, b2f], axis=1)[None], (P, L, HALF)
        )
    )

    if "nc" not in _cache:
        _cache["nc"] = _build_nc()
    nc = _cache["nc"]

    in_maps = []
    for c in range(NCORES):
        x0q = (h[c * BPC:(c + 1) * BPC] * XSCALE).astype(
            ml_dtypes.float8_e4m3
        )
        # partition-major: x0r[b, p, c, f] = x0q[b, c*P + p, f]
        x0r = np.ascontiguousarray(
            x0q.reshape(BPC, NT, P, F).transpose(0, 2, 1, 3)
        )
        in_maps.append({
            "x0": x0r,
            "bm": bmr,
            "w1": w1r,
            "w2": w2r,
            "b1": b1r,
            "b2": b2r,
            "ones": np.ones((1, P), dtype=np.float32),
        })

    trace = os.environ.get("KERNEL_TRACE") == "1"
    res = run_bass_kernel_spmd(
        nc, in_maps, core_ids=list(range(NCORES)), trace=trace
    )
    _cache["last_results"] = res
    # out comes back as [L, BPC, P, NT, F]; un-permute the node dim
    # (n = t * P + p) back to [L, BPC, N, F].
    outs = [
        np.ascontiguousarray(r["out"].transpose(0, 1, 3, 2, 4))
        .reshape(L, BPC, N, F)
        for r in res.results
    ]
    return np.concatenate(outs, axis=1)
